# revision 13
# baseline (speedup 1.0000x reference)
"""BBoxEncoder Trainium2 kernel.

Per ray r and BVH level l (8 levels) the reference gathers an embedding row
f = bbox_emb[history[r, l]] (8 corners x 32 dims), normalizes the ray's 16
sample points into the node's AABB, builds trilinear corner weights w[p, c]
and emits feat[r, l, p, d] = sum_c w[p, c] * f[c, d].

Device strategy (data-parallel over rays, 8 NeuronCores):
  - shard inp/history along axis 0; replicate one combined gather table:
    per node an fp16 row [256 emb fp16 (corner-permuted) | nmin(3 f32) |
    inv_extent(3 f32) viewed as 12 fp16 | 4 pad] = 544 B. One indirect-DMA
    gather per (tile, level) fetches embeddings + AABB together;
    inv_extent is precomputed on the host.
  - per 128-ray tile all compute runs on the DVE in fp16: normalized
    coords, factor products, 8 broadcast multiplies, and a 4-instruction
    paired add tree (fp16 packed 2x mode) into the accumulator.
  - the (128, 4096) fp16 accumulator is cast to f32 during the SWDGE store.

kernel(**inputs) takes FULL unsharded inputs and returns the FULL
(32768, 4096) float32 output.
"""

import numpy as np
from contextlib import ExitStack

import concourse.bass as bass
import concourse.tile as tile
from concourse import bacc, mybir
from concourse.bass import IndirectOffsetOnAxis, ts
from concourse.bass_utils import run_bass_kernel_spmd

F32 = mybir.dt.float32
F16 = mybir.dt.float16
I32 = mybir.dt.int32
AL = mybir.AluOpType

ENC_DEPTH = 8
N_POINTS = 16
ENC_DIM = 32

N_CORES = 8
N_RAYS = 32768
N_NODES = 65536

ROW16 = 272          # fp16 elems per table row (256 emb + 12 aabb + 4 pad)
ROW32 = ROW16 // 2   # f32 elems per row when bitcast

# reference (torch chunk) corner order as (bx, by, bz); emb columns are
# permuted on the host so device corner index c = bx*4+by*2+bz reads block c
_REF_CORNERS = [
    (0, 0, 0), (1, 0, 0), (0, 1, 0), (0, 0, 1),
    (1, 0, 1), (0, 1, 1), (1, 1, 0), (1, 1, 1),
]
PERM = [0] * 8
for _i, (_bx, _by, _bz) in enumerate(_REF_CORNERS):
    PERM[_bx * 4 + _by * 2 + _bz] = _i


def _emit(ctx, tc, io, n_shard, n_levels):
    nc = tc.nc
    P = 128
    L = n_levels
    n_tiles = n_shard // P
    PD = N_POINTS * ENC_DIM          # 512
    OUT_W = ENC_DEPTH * PD           # 4096

    inp_d = io["inp"].ap()           # (n_shard, 48) f32
    hist_d = io["hist"].ap()         # (n_shard, 8) int32
    table_d = io["table"]            # (N_NODES, ROW16) fp16
    out_d = io["out"].ap()           # (n_shard, 4096) f32

    ld = ctx.enter_context(tc.tile_pool(name="ld", bufs=3))
    gat = ctx.enter_context(tc.tile_pool(name="gat", bufs=3))
    wrk = ctx.enter_context(tc.tile_pool(name="wrk", bufs=2))
    mac = ctx.enter_context(tc.tile_pool(name="mac", bufs=1))
    frp = ctx.enter_context(tc.tile_pool(name="frp", bufs=4))
    acc_p = ctx.enter_context(tc.tile_pool(name="acc", bufs=3))

    for i in range(n_tiles):
        inp_t = ld.tile([P, 48], F32, tag="inp")
        nc.sync.dma_start(inp_t[:], inp_d[ts(i, P), :])
        hist_t = ld.tile([P, ENC_DEPTH], I32, tag="hist")
        nc.sync.dma_start(hist_t[:], hist_d[ts(i, P), :])

        f_t = gat.tile([P, L * ROW16], F16, tag="f")
        for l in range(L):
            nc.gpsimd.indirect_dma_start(
                out=f_t[:, l * ROW16:(l + 1) * ROW16],
                out_offset=None,
                in_=table_d.ap(),
                in_offset=IndirectOffsetOnAxis(ap=hist_t[:, l:l + 1], axis=0),
            )

        # --- normalized coords x[a, l, p] = clip((inp - nmin) * inv) ---
        nd_v = f_t[:].bitcast(F32).rearrange(
            "q (l e) -> q l e", e=ROW32)              # (q, L, 136)
        nmin_b = (nd_v[:, :, 128:131].transpose([0, 2, 1])
                  .unsqueeze(3).to_broadcast([P, 3, L, N_POINTS]))
        inv_b = (nd_v[:, :, 131:134].transpose([0, 2, 1])
                 .unsqueeze(3).to_broadcast([P, 3, L, N_POINTS]))
        inp_b = (inp_t[:].rearrange("q (p a) -> q p a", a=3)
                 .transpose([0, 2, 1])
                 .unsqueeze(2).to_broadcast([P, 3, L, N_POINTS]))

        xs_t = wrk.tile([P, 3 * L * N_POINTS], F32, tag="xs")
        xs_v = xs_t[:].rearrange("q (a l p) -> q a l p", a=3, p=N_POINTS)
        nc.vector.tensor_tensor(out=xs_v, in0=inp_b, in1=nmin_b,
                                op=AL.subtract)
        nc.vector.tensor_tensor(out=xs_v, in0=xs_v, in1=inv_b, op=AL.mult)

        # --- factors ft[s, a, l, p]: s=0 -> 1-x, s=1 -> x (clipped) ---
        ft_t = wrk.tile([P, 2 * 3 * L * N_POINTS], F16, tag="ft")
        ft_v = ft_t[:].rearrange(
            "q (s a l p) -> q s a l p", s=2, a=3, p=N_POINTS)
        nc.vector.tensor_scalar(
            out=ft_v[:, 1], in0=xs_v, scalar1=0.0, scalar2=1.0,
            op0=AL.max, op1=AL.min,
        )
        nc.vector.tensor_scalar(
            out=ft_v[:, 0], in0=ft_v[:, 1], scalar1=-1.0, scalar2=1.0,
            op0=AL.mult, op1=AL.add,
        )

        # --- corner weights w[x, y, z, (l p)] ---
        M = L * N_POINTS
        fx = (ft_v[:, :, 0].rearrange("q s l p -> q s (l p)")
              .unsqueeze(2).to_broadcast([P, 2, 2, M]))
        fy = (ft_v[:, :, 1].rearrange("q s l p -> q s (l p)")
              .unsqueeze(1).to_broadcast([P, 2, 2, M]))
        wxy_t = wrk.tile([P, 4 * M], F16, tag="wxy")
        wxy_v = wxy_t[:].rearrange("q (x y m) -> q x y m", x=2, y=2)
        nc.vector.tensor_tensor(out=wxy_v, in0=fx, in1=fy, op=AL.mult)

        # weights duplicated into d-pairs: wdup[x, y, z, (l p), pair] so the
        # MAC multiplies run in the DVE fp16 packed 2x mode
        wd_t = wrk.tile([P, 16 * M], F16, tag="wd")
        wd_v = wd_t[:].rearrange(
            "q (x y z m r) -> q x y z m r", x=2, y=2, z=2, r=2)
        wxy_p = (wxy_t[:].rearrange("q (xy m) -> q xy m", xy=4)
                 .unsqueeze(3).to_broadcast([P, 4, M, 2]))
        for z in range(2):
            fz = (ft_v[:, z, 2].rearrange("q l p -> q (l p)")
                  .unsqueeze(1).unsqueeze(3).to_broadcast([P, 4, M, 2]))
            nc.vector.tensor_tensor(
                out=wd_v[:, :, :, z].rearrange("q x y m r -> q (x y) m r"),
                in0=wxy_p, in1=fz, op=AL.mult)

        # --- 8-corner MAC: acc[l, p, d] = sum_c w[c, l, p] * f[c, l, d] ---
        # The scalar (ACT) engine replicates each gathered corner row over
        # the 16 points (fr_c, contiguous fp16) in parallel with DVE work;
        # the DVE multiplies then run fully packed at 2x. Paired adds fold
        # the 8 products into the accumulator.
        f_l = f_t[:].rearrange("q (l e) -> q l e", e=ROW16)
        LP = L * PD
        tU = mac.tile([P, 4 * LP], F16, tag="tU")
        tV = mac.tile([P, 4 * LP], F16, tag="tV")

        fr_list = []
        for c in range(8):
            fr = frp.tile([P, LP], F16, tag="fr")
            f_b = (f_l[:, :, c * ENC_DIM:(c + 1) * ENC_DIM]
                   .unsqueeze(2).to_broadcast([P, L, N_POINTS, ENC_DIM]))
            nc.scalar.copy(
                fr[:].rearrange("q (l p d) -> q l p d",
                                p=N_POINTS, d=ENC_DIM), f_b)
            fr_list.append(fr)

        def mul(dst_ap, c):
            bx, by, bz = (c >> 2) & 1, (c >> 1) & 1, c & 1
            w_b = (wd_v[:, bx, by, bz]
                   .unsqueeze(2).to_broadcast([P, M, N_POINTS, 2]))
            f_b = fr_list[c][:].rearrange("q (m e r) -> q m e r", m=M, r=2)
            nc.vector.tensor_tensor(
                out=dst_ap.rearrange("q (m e r) -> q m e r", m=M, r=2),
                in0=w_b, in1=f_b, op=AL.mult)

        def quads(t, which):      # (q, j, n) view of quads {0,2} or {1,3}
            v = t[:].rearrange("q (j k n) -> q j k n", j=2, k=2)
            return v[:, :, which]

        def halfv(t, h):          # (q, j, n) view of half h
            return t[:, h * 2 * LP:(h + 1) * 2 * LP].rearrange(
                "q (j n) -> q j n", j=2)

        acc_t = acc_p.tile([P, OUT_W], F16, tag="acc")
        acc_v = acc_t[:, :LP]
        for c in range(4):
            mul(tU[:, c * LP:(c + 1) * LP], c)
        nc.vector.tensor_tensor(out=halfv(tV, 0), in0=quads(tU, 0),
                                in1=quads(tU, 1), op=AL.add)
        for c in range(4, 8):
            mul(tU[:, (c - 4) * LP:(c - 3) * LP], c)
        nc.vector.tensor_tensor(out=halfv(tV, 1), in0=quads(tU, 0),
                                in1=quads(tU, 1), op=AL.add)
        nc.vector.tensor_tensor(out=halfv(tU, 0), in0=quads(tV, 0),
                                in1=quads(tV, 1), op=AL.add)
        nc.vector.tensor_tensor(out=acc_v, in0=tU[:, :LP],
                                in1=tU[:, LP:2 * LP], op=AL.add)

        if L < ENC_DEPTH:
            nc.gpsimd.memset(acc_t[:, LP:], 0.0)

        # SWDGE cast-DMA: fp16 SBUF -> f32 HBM
        nc.gpsimd.dma_start(out_d[ts(i, P), :], acc_t[:])


def build_program(n_shard, n_nodes, n_levels):
    nc = bacc.Bacc(
        "TRN2", target_bir_lowering=False, debug=False, enable_asserts=False
    )
    io = {
        "inp": nc.dram_tensor("inp", [n_shard, 48], F32, kind="ExternalInput"),
        "hist": nc.dram_tensor("hist", [n_shard, ENC_DEPTH], I32,
                               kind="ExternalInput"),
        "table": nc.dram_tensor("table", [n_nodes, ROW16], F16,
                                kind="ExternalInput"),
        "out": nc.dram_tensor("out", [n_shard, ENC_DEPTH * N_POINTS * ENC_DIM],
                              F32, kind="ExternalOutput"),
    }
    with tile.TileContext(nc) as tc, ExitStack() as ctx:
        _emit(ctx, tc, io, n_shard, n_levels)
    nc.compile()
    return nc


_CACHE = {}


def _get_program(n_shard, n_nodes, n_levels):
    key = (n_shard, n_nodes, n_levels)
    if key not in _CACHE:
        _CACHE[key] = build_program(n_shard, n_nodes, n_levels)
    return _CACHE[key]


_MARSHAL = {}


def make_table(bbox_emb, nodes_min, nodes_max):
    n_nodes = bbox_emb.shape[0]
    emb16 = (bbox_emb.astype(np.float32, copy=False)
             .reshape(n_nodes, 8, ENC_DIM)[:, PERM, :]
             .reshape(n_nodes, 256).astype(np.float16))
    nmin = nodes_min.astype(np.float32, copy=False)
    ext = nodes_max.astype(np.float32, copy=False) - nmin
    ext = np.where(ext == 0, np.float32(0.5), ext)
    inv = (np.float32(1.0) / ext).astype(np.float32)
    aabb = np.ascontiguousarray(
        np.concatenate([nmin, inv], axis=1))        # (n, 6) f32
    table = np.zeros((n_nodes, ROW16), np.float16)
    table[:, :256] = emb16
    table[:, 256:268] = aabb.view(np.float16)
    return np.ascontiguousarray(table)


def make_in_maps(inp, history, bbox_emb, nodes_min, nodes_max,
                 n_cores=N_CORES):
    key = (id(inp), id(history), id(bbox_emb), id(nodes_min), id(nodes_max))
    if _MARSHAL.get("key") != key:
        n_rays = inp.shape[0]
        _MARSHAL.clear()
        _MARSHAL["key"] = key
        _MARSHAL["refs"] = (inp, history, bbox_emb, nodes_min, nodes_max)
        _MARSHAL["inp"] = np.ascontiguousarray(
            inp.reshape(n_rays, 48).astype(np.float32, copy=False))
        _MARSHAL["hist"] = np.ascontiguousarray(
            history[:, :ENC_DEPTH].astype(np.int32, copy=False))
        _MARSHAL["table"] = make_table(bbox_emb, nodes_min, nodes_max)
    inp_f = _MARSHAL["inp"]
    hist8 = _MARSHAL["hist"]
    table = _MARSHAL["table"]
    n_rays = inp_f.shape[0]
    shard = n_rays // n_cores
    mk = ("maps", n_cores)
    if mk not in _MARSHAL:
        in_maps = []
        for c in range(n_cores):
            sl = slice(c * shard, (c + 1) * shard)
            in_maps.append({
                "inp": inp_f[sl],
                "hist": hist8[sl],
                "table": table,
            })
        _MARSHAL[mk] = in_maps
    return _MARSHAL[mk], shard, table.shape[0]


_FAST = {}
_REPL = frozenset({"table"})


def _fast_entry(nc, n_cores):
    """Build (once) the sharded jit for nc: replicated tensors ship a single
    copy, zero output buffers ride as cached non-donated device args."""
    import jax
    from jax.sharding import Mesh, PartitionSpec as P
    from jax.experimental.shard_map import shard_map
    from concourse import bass2jax

    key = id(nc)
    if key in _FAST:
        return _FAST[key]
    bass2jax.install_neuronx_cc_hook()
    part_name = nc.partition_id_tensor.name if nc.partition_id_tensor else None
    in_names, out_names, out_avals = [], [], []
    for alloc in nc.m.functions[0].allocations:
        if not isinstance(alloc, mybir.MemoryLocationSet):
            continue
        name = alloc.memorylocations[0].name
        if alloc.kind == "ExternalInput":
            if name != part_name:
                in_names.append(name)
        elif alloc.kind == "ExternalOutput":
            out_names.append(name)
            out_avals.append(jax.core.ShapedArray(
                tuple(alloc.tensor_shape), mybir.dt.np(alloc.dtype)))
    all_names = list(in_names) + list(out_names)
    if part_name is not None:
        all_names.append(part_name)

    def _body(*args):
        operands = list(args)
        if part_name is not None:
            operands.append(bass2jax.partition_id_tensor())
        return tuple(bass2jax._bass_exec_p.bind(
            *operands,
            out_avals=tuple(out_avals),
            in_names=tuple(all_names),
            out_names=tuple(out_names),
            lowering_input_output_aliases=(),
            sim_require_finite=False,
            sim_require_nnan=False,
            nc=nc,
        ))

    devices = jax.devices()[:n_cores]
    assert len(devices) == n_cores, "not enough devices"
    mesh = Mesh(np.asarray(devices), ("core",))
    in_specs = tuple(
        P() if nm in _REPL else P("core") for nm in in_names
    ) + (P("core"),) * len(out_names)
    fn = jax.jit(shard_map(
        _body, mesh=mesh, in_specs=in_specs,
        out_specs=(P("core"),) * len(out_names), check_rep=False,
    ))
    entry = {"fn": fn, "in_names": in_names, "out_names": out_names,
             "out_avals": out_avals, "mesh": mesh, "placed": {}}
    _FAST[key] = entry
    return entry


def _fast_args(nc, in_maps):
    """Device placements for one execution; replicated + sharded inputs and
    the zero output buffers are cached keyed by source array identity."""
    import jax
    from jax.sharding import NamedSharding, PartitionSpec as P

    n_cores = len(in_maps)
    e = _fast_entry(nc, n_cores)
    mesh, placed = e["mesh"], e["placed"]
    args = []
    for nm in e["in_names"]:
        if nm in _REPL:
            src = in_maps[0][nm]
            spec = P()
        else:
            src = [m[nm] for m in in_maps]
            spec = P("core")
        pk = (nm,) + tuple(id(s) for s in (src if isinstance(src, list)
                                           else [src]))
        if pk not in placed:
            for k in [k for k in placed if k[0] == nm]:
                del placed[k]
            host = (np.concatenate([np.asarray(s) for s in src], axis=0)
                    if isinstance(src, list) else np.asarray(src))
            placed[pk] = (src, jax.device_put(host, NamedSharding(mesh, spec)))
        args.append(placed[pk][1])
    zk = ("zeros",)
    if zk not in placed:
        placed[zk] = (None, [
            jax.device_put(
                np.zeros((n_cores * av.shape[0], *av.shape[1:]), av.dtype),
                NamedSharding(mesh, P("core")))
            for av in e["out_avals"]
        ])
    args.extend(placed[zk][1])
    return e, args


def _run_fast(nc, in_maps):
    e, args = _fast_args(nc, in_maps)
    outs = e["fn"](*args)
    return {nm: np.asarray(outs[i]) for i, nm in enumerate(e["out_names"])}


def measure_hw_exec_ns(inputs, n1=2, n2=18, rounds=7):
    """Marginal per-execution device time via pipelined dispatch: issue n
    async executions with device-resident inputs, block once; the slope
    between two batch sizes cancels the per-call RPC/launch overhead.
    Uses min-of-batch-times across rounds to reject scheduler hiccups."""
    import time
    import jax

    n_levels = int(min(int(np.asarray(inputs["depth"]).max()), ENC_DEPTH))
    in_maps, shard, n_nodes = make_in_maps(
        np.asarray(inputs["inp"]), np.asarray(inputs["history"]),
        np.asarray(inputs["bbox_emb"]), np.asarray(inputs["nodes_min"]),
        np.asarray(inputs["nodes_max"]))
    nc = _get_program(shard, n_nodes, n_levels)
    e, args = _fast_args(nc, in_maps)
    fn = e["fn"]
    jax.block_until_ready(fn(*args))  # warm (compile, NEFF load)

    def batch(n):
        t0 = time.time()
        outs = [fn(*args) for _ in range(n)]
        jax.block_until_ready(outs)
        return time.time() - t0

    batch(2)
    t1s, t2s = [], []
    for _ in range(rounds):
        t1s.append(batch(n1))
        t2s.append(batch(n2))
    # min over rounds estimates the uncontended time for each batch size;
    # their difference is the marginal device time of (n2 - n1) executions
    slope = (min(t2s) - min(t1s)) / (n2 - n1)
    if slope <= 0:
        slopes = sorted((b - a) / (n2 - n1) for a, b in zip(t1s, t2s))
        slope = slopes[len(slopes) // 2]
    return max(slope, 1e-9) * 1e9


def kernel(inp, history, depth, bbox_emb, nodes_min, nodes_max):
    inp = np.asarray(inp)
    history = np.asarray(history)
    depth = np.asarray(depth)
    bbox_emb = np.asarray(bbox_emb)
    nodes_min = np.asarray(nodes_min)
    nodes_max = np.asarray(nodes_max)

    n_rays = inp.shape[0]
    n_levels = int(min(int(depth.max()), ENC_DEPTH)) if depth.size else 0
    if n_levels <= 0:
        return np.zeros((n_rays, ENC_DEPTH * N_POINTS * ENC_DIM), np.float32)
    in_maps, shard, n_nodes = make_in_maps(
        inp, history, bbox_emb, nodes_min, nodes_max
    )
    nc = _get_program(shard, n_nodes, n_levels)
    try:
        import jax
        # the fast path must not silently fall into the CPU CoreSim
        # lowering — only use it on real accelerator devices
        assert jax.devices()[0].platform != "cpu"
        return _run_fast(nc, in_maps)["out"]
    except Exception:
        res = run_bass_kernel_spmd(nc, in_maps, core_ids=list(range(N_CORES)))
        return np.concatenate([r["out"] for r in res.results], axis=0)
